# revision 12
# baseline (speedup 1.0000x reference)
"""NVFP4 fake-quant SwiGLU MLP on 8 Trainium2 NeuronCores.

Sharding: data-parallel over tokens for the matmuls (each core computes 1024
of the 8192 tokens end-to-end); weight *quantization* is sharded Megatron-style
(each core fake-quants 1/8 of each weight) and the quantized bf16 weights are
AllGathered in 128-row stripes so the gate/up matmuls can start as soon as the
first stripe lands. No other collective is needed: the final output is
token-sharded and concatenated on the host.

Math: fake-quant values q*sc8 are exactly representable in bf16 (q: 2 sig
bits, sc8: e4m3fn 4 sig bits), so all three matmuls run at bf16 PE peak and
the global scales 1/(gs_a*gs_w) are applied to the f32 outputs. e2m1 and
e4m3fn round-to-nearest are computed with custom DVE ops (Veltkamp splitting
for the normal ranges + magic-constant fixed-point rounds for the subnormal
ranges); the scale-clip and subnormal-round stages are fused into one DVE
pass since a fused spec still streams at 1 elem/lane/cycle.

Hidden states bounce through DRAM as [T,I] and come back transposed via
32 big DMA transposes into a resident SBUF [I,T] copy, so the down matmul
runs back-to-back on the PE right after the gate/up chunks finish.
"""
import numpy as np

import concourse.bass as bass
import concourse.mybir as mybir
import concourse.tile as tile
from concourse import bacc
from concourse.bass_utils import run_bass_kernel_spmd
from concourse.dve_spec import (
    Spec, Src0, Src1, C0, C1, C2, C3, One, Zero, lower, maxx, minn, select, sq,
    _has_src1, _spill_c3_to_src1,
)
import concourse.dve_ops as dve_ops_mod
from concourse.dve_ops import DveOp, OPS
from concourse.dve_uop import DveOpSpec

F32 = mybir.dt.float32
BF16 = mybir.dt.bfloat16
ALU = mybir.AluOpType
AX = mybir.AxisListType
AF = mybir.ActivationFunctionType

B, S, H, I = 4, 2048, 1024, 4096
NCORES = 8
T = B * S
T_LOC = T // NCORES      # 1024 tokens per core
I_SH = I // NCORES       # 512  gate/up rows per core (quant shard)
HO_SH = H // NCORES      # 128  down rows per core (quant shard)
NSTRIPE = I_SH // 128    # 4 AllGather stripes per gate/up weight

VELT_E2M1 = float(2**22 + 1)
MAGIC_E2M1 = float(3 * 2**21)
VELT_E4M3 = float(2**20 + 1)
MAGIC_E4M3 = float(2**14)
TH_E4M3 = float(2**-6)

# ---------------------------------------------------------------- custom ops


def _register(name, spec, subdim=False):
    for op in OPS:
        if op.name == name:
            return op
    idx = len(OPS)
    opcode = dve_ops_mod._CUSTOM_DVE_ROW_BASE + idx
    assert opcode < 0x20, "custom DVE row overflow"
    shas = {}
    for ver in ("v3", "v4"):
        shas[ver] = DveOpSpec(
            name=name, opcode=opcode, uops=lower(spec, ver=ver),
            rd1_en=_has_src1(spec),
        ).sha(ver)
    op = DveOp(name, spec, subdim=subdim, uops_sha=shas)
    OPS.append(op)
    dve_ops_mod._SUB_OPCODE_FOR_NAME[name] = opcode
    dve_ops_mod.CUSTOM_DVE_SPECS[name] = spec
    return op


def _ref_clip_subnorm(in0, in1, s0, s1, imm2):
    m = (in0.astype(np.float32) * in1.astype(np.float32)).astype(np.float32)
    m = np.minimum(np.maximum(m, np.float32(-s0)), np.float32(s0))
    u = (m + np.float32(s1)).astype(np.float32)
    v = (u - np.float32(s1)).astype(np.float32)
    return np.where((m * m).astype(np.float32) < 1.0, v, m).astype(np.float32)


def _ref_velt_scale(in0, in1, s0, s1, imm2):
    t = in0.astype(np.float32)
    gam = (t * np.float32(s0)).astype(np.float32)
    delta = (t - gam).astype(np.float32)
    hi = (gam + delta).astype(np.float32)
    return (hi * in1.astype(np.float32)).astype(np.float32)


def _ref_e4m3(in0, in1, s0, s1, imm2):
    cap = in1.reshape(in1.shape[0], 1).astype(np.float32)
    t = np.minimum(in0.astype(np.float32), cap)
    gam = (t * np.float32(s0)).astype(np.float32)
    delta = (t - gam).astype(np.float32)
    hi = (gam + delta).astype(np.float32)
    u = (t + np.float32(s1)).astype(np.float32)
    v = (u - np.float32(s1)).astype(np.float32)
    return np.where(t < np.float32(imm2), v, hi).astype(np.float32)


# fused: m = clip(src*r2, +-6); subnormal (|m|<1) rounds to multiples of 0.5
_m = minn(maxx(Src0 * Src1, Zero - C0), C0)
_mv = (_m + C1) - C1
OP_CLIP_SUBNORM = _register(
    "NVFP4_CLIP_SUBNORM_ANT",
    Spec(body=select(sq(_m) < One, _mv, _m), reference=_ref_clip_subnorm),
)
_gam = Src0 * C0
_hi = _gam + (Src0 - _gam)
OP_VELT_SCALE = _register(
    "NVFP4_VELT_SCALE_ANT",
    Spec(body=_hi * Src1, reference=_ref_velt_scale),
)
_t = minn(Src0, C3)
_gam4 = _t * C0
_hi4 = _gam4 + (_t - _gam4)
_v4 = (_t + C1) - C1
OP_E4M3 = _register(
    "NVFP4_E4M3_ANT",
    Spec(body=_spill_c3_to_src1(select(_t < C2, _v4, _hi4)), reference=_ref_e4m3),
)


def quantize_tile(nc, work, src_f32, out_bf16, n, gs, c448_col):
    """src_f32 [128, n] (true values, 16-blocks on free dim) -> out_bf16 = q*sc8."""
    nblk = n // 16
    gs = float(np.float32(gs))
    src3 = src_f32.rearrange("p (b s) -> p b s", s=16)
    amax = work.tile([128, nblk], F32, tag="q_amax")
    nc.vector.tensor_reduce(
        out=amax[:], in_=src3, axis=AX.X, op=ALU.max, apply_absolute_value=True
    )
    t1 = work.tile([128, nblk], F32, tag="q_t1")
    nc.vector.tensor_scalar(
        out=t1[:], in0=amax[:], scalar1=float(np.float32(1.0 / 6.0)), scalar2=gs,
        op0=ALU.mult, op1=ALU.mult,
    )
    sc8 = work.tile([128, nblk], F32, tag="q_sc8")
    nc.vector._custom_dve(
        OP_E4M3, out=sc8[:], in0=t1[:], in1=c448_col,
        s0=VELT_E4M3, s1=MAGIC_E4M3, imm2=TH_E4M3,
    )
    r = work.tile([128, nblk], F32, tag="q_r")
    nc.vector.reciprocal(r[:], sc8[:])
    r2 = work.tile([128, nblk], F32, tag="q_r2")
    nc.vector.tensor_scalar(
        out=r2[:], in0=r[:], scalar1=gs, scalar2=1e38,
        op0=ALU.mult, op1=ALU.min,
    )
    pp = work.tile([128, n], F32, tag="q_pp")
    pp3 = pp[:].rearrange("p (b s) -> p b s", s=16)
    r2b = r2[:].unsqueeze(-1).broadcast_to([128, nblk, 16])
    nc.vector._custom_dve(
        OP_CLIP_SUBNORM, out=pp3, in0=src3, in1=r2b, s0=6.0, s1=MAGIC_E2M1,
    )
    sc8b = sc8[:].unsqueeze(-1).broadcast_to([128, nblk, 16])
    out3 = out_bf16.rearrange("p (b s) -> p b s", s=16)
    nc.vector._custom_dve(OP_VELT_SCALE, out=out3, in0=pp3, in1=sc8b, s0=VELT_E2M1)


# ---------------------------------------------------------------- program


def build_program(gs_x, gs_gw, gs_uw, gs_dw, gs_h):
    gs_x, gs_gw, gs_uw, gs_dw, gs_h = (
        np.float32(gs_x), np.float32(gs_gw), np.float32(gs_uw),
        np.float32(gs_dw), np.float32(gs_h),
    )
    s_gate = float(np.float32(1.0) / np.float32(gs_x * gs_gw))
    s_up = float(np.float32(1.0) / np.float32(gs_x * gs_uw))
    s_down = float(np.float32(1.0) / np.float32(gs_h * gs_dw))

    nc = bacc.Bacc("TRN2", num_devices=NCORES, debug=False)
    x_in = nc.dram_tensor("x_slice", [T_LOC, H], F32, kind="ExternalInput")
    gw_in = nc.dram_tensor("gw_slice", [I_SH, H], F32, kind="ExternalInput")
    uw_in = nc.dram_tensor("uw_slice", [I_SH, H], F32, kind="ExternalInput")
    dw_in = nc.dram_tensor("dw_slice", [HO_SH, I], F32, kind="ExternalInput")
    out_d = nc.dram_tensor("out_slice", [T_LOC, H], F32, kind="ExternalOutput")

    RG = [list(range(NCORES))]
    NT = T_LOC // 128            # 8 token tiles
    NHT = H // 128               # 8 contraction tiles for gate/up
    NCH = I // 512               # 8 I-chunks of 512
    NB = I // 128                # 32 hidden column blocks

    with tile.TileContext(nc) as tc:
        with (
            tc.tile_pool(name="dram", bufs=1, space="DRAM") as dpool,
            tc.tile_pool(name="const", bufs=1) as cpool,
            tc.tile_pool(name="xt", bufs=1) as xtpool,
        ):
            # DRAM staging: local quant shards + gathered weights.
            # guw_loc rows 0-511 = gate shard, 512-1023 = up shard, so ONE
            # AllGather delivers both with each rank's contribution
            # contiguous: chunk k of phase C = core k's I-shard, natural
            # I order.
            guw_loc = dpool.tile([2 * I_SH, H], BF16)
            dw_loc = dpool.tile([HO_SH, I], BF16)
            guw_g = dpool.tile([2 * I_SH * NCORES, H], BF16, addr_space="Shared")
            dw_g = dpool.tile([H, I], BF16, addr_space="Shared")
            xq_d = dpool.tile([T_LOC, H], BF16)
            hq_d = dpool.tile([T_LOC, I], BF16)

            c448 = cpool.tile([128, 1], F32)
            nc.vector.memset(c448[:], 448.0)

            # xqT[128h, ht, tok]  (resident through phase C)
            xqT = xtpool.tile([128, NHT, T_LOC], BF16)

            # ---- Phase A/B: quantize weight stripes + x, kick striped AGs
            with (
                tc.tile_pool(name="raw", bufs=4) as raw,
                tc.tile_pool(name="qout", bufs=4) as qout,
                tc.tile_pool(name="workAB", bufs=3) as workAB,
            ):
                def quant_stripe(src, dst_loc, row0, cols, gsw):
                    wt = raw.tile([128, cols], F32, tag="raw")
                    nc.scalar.dma_start(wt[:], src[row0:row0 + 128, :])
                    wq = qout.tile([128, cols], BF16, tag="qout")
                    quantize_tile(nc, workAB, wt[:], wq[:], cols, gsw, c448[:])
                    nc.scalar.dma_start(dst_loc, wq[:])

                # gate+up shards into the combined tile, then one AllGather
                for j in range(NSTRIPE):
                    quant_stripe(gw_in, guw_loc[j * 128:(j + 1) * 128, :],
                                 j * 128, H, gs_gw)
                for j in range(NSTRIPE):
                    quant_stripe(uw_in,
                                 guw_loc[I_SH + j * 128:I_SH + (j + 1) * 128, :],
                                 j * 128, H, gs_uw)
                nc.gpsimd.collective_compute(
                    "AllGather", ALU.bypass, replica_groups=RG,
                    ins=[guw_loc[:]], outs=[guw_g[:]],
                )

                # x: quantize per 128-token tile, stage via DRAM
                for t in range(NT):
                    xt = raw.tile([128, H], F32, tag="raw")
                    nc.scalar.dma_start(xt[:], x_in[t * 128:(t + 1) * 128, :])
                    xq = qout.tile([128, H], BF16, tag="qout")
                    quantize_tile(nc, workAB, xt[:], xq[:], H, gs_x, c448[:])
                    nc.scalar.dma_start(xq_d[t * 128:(t + 1) * 128, :], xq[:])
                # one big single-writer transpose per ht (simple dep graph:
                # each matmul lhsT read falls inside exactly one writer)
                for ht in range(NHT):
                    nc.sync.dma_start_transpose(
                        xqT[:, ht, :], xq_d[:, ht * 128:(ht + 1) * 128]
                    )

                # down weight shard: quantize in H-sized column pieces
                for p in range(I // H):
                    dwt = raw.tile([128, H], F32, tag="raw")
                    nc.scalar.dma_start(dwt[:], dw_in[:, p * H:(p + 1) * H])
                    dwq = qout.tile([128, H], BF16, tag="qout")
                    quantize_tile(nc, workAB, dwt[:], dwq[:], H, gs_dw, c448[:])
                    nc.scalar.dma_start(dw_loc[:, p * H:(p + 1) * H], dwq[:])
                nc.gpsimd.collective_compute(
                    "AllGather", ALU.bypass, replica_groups=RG,
                    ins=[dw_loc[:]], outs=[dw_g[:]],
                )

            # resident transposed hidden + down weights (filled later)
            with (
                tc.tile_pool(name="hqt", bufs=1) as hqtpool,
                tc.tile_pool(name="dwt", bufs=1) as dwtpool,
                tc.tile_pool(name="wstr", bufs=2) as wstr,
                tc.tile_pool(name="psgu", bufs=3, space="PSUM") as psgu,
                tc.tile_pool(name="hwork", bufs=2) as hwork,
                tc.tile_pool(name="workC", bufs=2) as workC,
                tc.tile_pool(name="hqout", bufs=2) as hqout,
                tc.tile_pool(name="pso", bufs=1, space="PSUM") as pso,
                tc.tile_pool(name="obuf", bufs=2) as obuf,
            ):
                hqT = hqtpool.tile([128, NB, T_LOC], BF16)
                dwT = dwtpool.tile([128, NB, H], BF16)

                # ---- Phase C: gate/up matmuls + SwiGLU + hidden quant
                # weight-chunk transposes are emitted one chunk ahead so
                # they overlap the previous chunk's matmuls on the Sync
                # queue instead of queueing behind its hq stores
                wtiles = []

                def emit_chunk_transposes(k):
                    gwT = wstr.tile([128, NHT, 512], BF16, tag="gwT")
                    uwT = wstr.tile([128, NHT, 512], BF16, tag="uwT")
                    g0 = k * 2 * I_SH
                    for ht in range(NHT):
                        nc.sync.dma_start_transpose(
                            gwT[:, ht, :],
                            guw_g[g0:g0 + 512, ht * 128:(ht + 1) * 128],
                        )
                        nc.sync.dma_start_transpose(
                            uwT[:, ht, :],
                            guw_g[g0 + I_SH:g0 + I_SH + 512,
                                  ht * 128:(ht + 1) * 128],
                        )
                    wtiles.append((gwT, uwT))

                emit_chunk_transposes(0)
                for k in range(NCH):
                    if k + 1 < NCH:
                        emit_chunk_transposes(k + 1)
                    gwT, uwT = wtiles[k]
                    if k == 5:
                        # down-weight transposes (natural I order); AG_dw
                        # has landed by now, phase D needs them much later
                        for b in range(NB):
                            nc.sync.dma_start_transpose(
                                dwT[:, b, :], dw_g[:, b * 128:(b + 1) * 128]
                            )
                    for tch in range(NT):
                        pg = psgu.tile([128, 512], F32, tag="pg")
                        pu = psgu.tile([128, 512], F32, tag="pu")
                        for ht in range(NHT):
                            lhsT = xqT[:, ht, tch * 128:(tch + 1) * 128]
                            nc.tensor.matmul(
                                pg[:], lhsT, gwT[:, ht, :],
                                start=(ht == 0), stop=(ht == NHT - 1),
                            )
                            nc.tensor.matmul(
                                pu[:], lhsT, uwT[:, ht, :],
                                start=(ht == 0), stop=(ht == NHT - 1),
                            )
                        sil = hwork.tile([128, 512], F32, tag="sil")
                        nc.scalar.activation(sil[:], pg[:], AF.Silu, scale=s_gate)
                        htr = hwork.tile([128, 512], F32, tag="htr")
                        nc.vector.scalar_tensor_tensor(
                            out=htr[:], in0=sil[:], scalar=s_up, in1=pu[:],
                            op0=ALU.mult, op1=ALU.mult,
                        )
                        hq = hqout.tile([128, 512], BF16, tag="hq")
                        quantize_tile(nc, workC, htr[:], hq[:], 512, gs_h, c448[:])
                        nc.sync.dma_start(
                            hq_d[tch * 128:(tch + 1) * 128,
                                 k * 512:(k + 1) * 512],
                            hq[:],
                        )
                    # this chunk's hidden columns transpose into resident
                    # hqT as soon as its stores land
                    for b in range(k * 4, k * 4 + 4):
                        nc.sync.dma_start_transpose(
                            hqT[:, b, :], hq_d[:, b * 128:(b + 1) * 128]
                        )

                # ---- Phase D: down matmul + output scale
                for tch in range(NT):
                    po = pso.tile([128, H], F32, tag="po")
                    for b in range(NB):
                        lhsT = hqT[:, b, tch * 128:(tch + 1) * 128]
                        nc.tensor.matmul(
                            po[:, 0:512], lhsT, dwT[:, b, 0:512],
                            start=(b == 0), stop=(b == NB - 1),
                        )
                        nc.tensor.matmul(
                            po[:, 512:1024], lhsT, dwT[:, b, 512:1024],
                            start=(b == 0), stop=(b == NB - 1),
                        )
                    ob = obuf.tile([128, H], F32, tag="ob")
                    nc.scalar.activation(ob[:], po[:], AF.Copy, scale=s_down)
                    nc.scalar.dma_start(
                        out_d[tch * 128:(tch + 1) * 128, :], ob[:]
                    )

    nc.finalize()
    return nc


_PROG_CACHE = {}
TRACE = False          # set by test.py to capture an NTFF profile
LAST_EXEC_NS = None
LAST_RESULTS = None


def kernel(x, gate_w, up_w, down_w, s_in, s_in_down):
    x = np.ascontiguousarray(x, dtype=np.float32)
    gate_w = np.ascontiguousarray(gate_w, dtype=np.float32)
    up_w = np.ascontiguousarray(up_w, dtype=np.float32)
    down_w = np.ascontiguousarray(down_w, dtype=np.float32)
    gs_x = np.float32(np.asarray(s_in).reshape(-1)[0])
    gs_h = np.float32(np.asarray(s_in_down).reshape(-1)[0])
    FM = np.float32(448.0 * 6.0)
    gs_gw = np.float32(FM / np.abs(gate_w).max())
    gs_uw = np.float32(FM / np.abs(up_w).max())
    gs_dw = np.float32(FM / np.abs(down_w).max())

    key = tuple(float(v) for v in (gs_x, gs_gw, gs_uw, gs_dw, gs_h))
    if key not in _PROG_CACHE:
        _PROG_CACHE.clear()
        _PROG_CACHE[key] = build_program(*key)
    nc = _PROG_CACHE[key]

    xf = x.reshape(T, H)
    in_maps = []
    for c in range(NCORES):
        in_maps.append({
            "x_slice": np.ascontiguousarray(xf[c * T_LOC:(c + 1) * T_LOC]),
            "gw_slice": np.ascontiguousarray(gate_w[c * I_SH:(c + 1) * I_SH]),
            "uw_slice": np.ascontiguousarray(up_w[c * I_SH:(c + 1) * I_SH]),
            "dw_slice": np.ascontiguousarray(down_w[c * HO_SH:(c + 1) * HO_SH]),
        })
    global LAST_EXEC_NS, LAST_RESULTS
    res = run_bass_kernel_spmd(
        nc, in_maps, core_ids=list(range(NCORES)), trace=TRACE
    )
    LAST_EXEC_NS = res.exec_time_ns
    LAST_RESULTS = res
    out = np.concatenate([r["out_slice"] for r in res.results], axis=0)
    return out.reshape(B, S, H).astype(np.float32)


if __name__ == "__main__":
    rng = np.random.default_rng(0)
    inputs = dict(
        x=rng.standard_normal((B, S, H), dtype=np.float32),
        gate_w=0.05 * rng.standard_normal((I, H), dtype=np.float32),
        up_w=0.05 * rng.standard_normal((I, H), dtype=np.float32),
        down_w=0.05 * rng.standard_normal((H, I), dtype=np.float32),
        s_in=np.array([700.0], dtype=np.float32),
        s_in_down=np.array([800.0], dtype=np.float32),
    )
    out = kernel(**inputs)
    print("kernel output", out.shape, out.dtype, np.abs(out).max())


# revision 13
# speedup vs baseline: 1.0447x; 1.0447x over previous
"""NVFP4 fake-quant SwiGLU MLP on 8 Trainium2 NeuronCores.

Sharding: data-parallel over tokens for the matmuls (each core computes 1024
of the 8192 tokens end-to-end); weight *quantization* is sharded Megatron-style
(each core fake-quants 1/8 of each weight) and the quantized bf16 weights are
AllGathered in 128-row stripes so the gate/up matmuls can start as soon as the
first stripe lands. No other collective is needed: the final output is
token-sharded and concatenated on the host.

Math: fake-quant values q*sc8 are exactly representable in bf16 (q: 2 sig
bits, sc8: e4m3fn 4 sig bits), so all three matmuls run at bf16 PE peak and
the global scales 1/(gs_a*gs_w) are applied to the f32 outputs. e2m1 and
e4m3fn round-to-nearest are computed with custom DVE ops (Veltkamp splitting
for the normal ranges + magic-constant fixed-point rounds for the subnormal
ranges); the scale-clip and subnormal-round stages are fused into one DVE
pass since a fused spec still streams at 1 elem/lane/cycle.

Hidden states bounce through DRAM as [T,I] and come back transposed via
32 big DMA transposes into a resident SBUF [I,T] copy, so the down matmul
runs back-to-back on the PE right after the gate/up chunks finish.
"""
import numpy as np

import concourse.bass as bass
import concourse.mybir as mybir
import concourse.tile as tile
from concourse import bacc
from concourse.bass_utils import run_bass_kernel_spmd
from concourse.dve_spec import (
    Spec, Src0, Src1, C0, C1, C2, C3, One, Zero, lower, maxx, minn, select, sq,
    _has_src1, _spill_c3_to_src1,
)
import concourse.dve_ops as dve_ops_mod
from concourse.dve_ops import DveOp, OPS
from concourse.dve_uop import DveOpSpec

F32 = mybir.dt.float32
BF16 = mybir.dt.bfloat16
ALU = mybir.AluOpType
AX = mybir.AxisListType
AF = mybir.ActivationFunctionType

B, S, H, I = 4, 2048, 1024, 4096
NCORES = 8
T = B * S
T_LOC = T // NCORES      # 1024 tokens per core
I_SH = I // NCORES       # 512  gate/up rows per core (quant shard)
HO_SH = H // NCORES      # 128  down rows per core (quant shard)
NSTRIPE = I_SH // 128    # 4 AllGather stripes per gate/up weight

VELT_E2M1 = float(2**22 + 1)
MAGIC_E2M1 = float(3 * 2**21)
VELT_E4M3 = float(2**20 + 1)
MAGIC_E4M3 = float(2**14)
TH_E4M3 = float(2**-6)

# ---------------------------------------------------------------- custom ops


def _register(name, spec, subdim=False):
    for op in OPS:
        if op.name == name:
            return op
    idx = len(OPS)
    opcode = dve_ops_mod._CUSTOM_DVE_ROW_BASE + idx
    assert opcode < 0x20, "custom DVE row overflow"
    shas = {}
    for ver in ("v3", "v4"):
        shas[ver] = DveOpSpec(
            name=name, opcode=opcode, uops=lower(spec, ver=ver),
            rd1_en=_has_src1(spec),
        ).sha(ver)
    op = DveOp(name, spec, subdim=subdim, uops_sha=shas)
    OPS.append(op)
    dve_ops_mod._SUB_OPCODE_FOR_NAME[name] = opcode
    dve_ops_mod.CUSTOM_DVE_SPECS[name] = spec
    return op


def _ref_clip_subnorm(in0, in1, s0, s1, imm2):
    m = (in0.astype(np.float32) * in1.astype(np.float32)).astype(np.float32)
    m = np.minimum(np.maximum(m, np.float32(-s0)), np.float32(s0))
    u = (m + np.float32(s1)).astype(np.float32)
    v = (u - np.float32(s1)).astype(np.float32)
    return np.where((m * m).astype(np.float32) < 1.0, v, m).astype(np.float32)


def _ref_velt_scale(in0, in1, s0, s1, imm2):
    t = in0.astype(np.float32)
    gam = (t * np.float32(s0)).astype(np.float32)
    delta = (t - gam).astype(np.float32)
    hi = (gam + delta).astype(np.float32)
    return (hi * in1.astype(np.float32)).astype(np.float32)


def _ref_e4m3(in0, in1, s0, s1, imm2):
    cap = in1.reshape(in1.shape[0], 1).astype(np.float32)
    t = np.minimum(in0.astype(np.float32), cap)
    gam = (t * np.float32(s0)).astype(np.float32)
    delta = (t - gam).astype(np.float32)
    hi = (gam + delta).astype(np.float32)
    u = (t + np.float32(s1)).astype(np.float32)
    v = (u - np.float32(s1)).astype(np.float32)
    return np.where(t < np.float32(imm2), v, hi).astype(np.float32)


# fused: m = clip(src*r2, +-6); subnormal (|m|<1) rounds to multiples of 0.5
_m = minn(maxx(Src0 * Src1, Zero - C0), C0)
_mv = (_m + C1) - C1
OP_CLIP_SUBNORM = _register(
    "NVFP4_CLIP_SUBNORM_ANT",
    Spec(body=select(sq(_m) < One, _mv, _m), reference=_ref_clip_subnorm),
)
_gam = Src0 * C0
_hi = _gam + (Src0 - _gam)
OP_VELT_SCALE = _register(
    "NVFP4_VELT_SCALE_ANT",
    Spec(body=_hi * Src1, reference=_ref_velt_scale),
)
_t = minn(Src0, C3)
_gam4 = _t * C0
_hi4 = _gam4 + (_t - _gam4)
_v4 = (_t + C1) - C1
OP_E4M3 = _register(
    "NVFP4_E4M3_ANT",
    Spec(body=_spill_c3_to_src1(select(_t < C2, _v4, _hi4)), reference=_ref_e4m3),
)


def quantize_tile(nc, work, src_f32, out_bf16, n, gs, c448_col):
    """src_f32 [128, n] (true values, 16-blocks on free dim) -> out_bf16 = q*sc8."""
    nblk = n // 16
    gs = float(np.float32(gs))
    src3 = src_f32.rearrange("p (b s) -> p b s", s=16)
    amax = work.tile([128, nblk], F32, tag="q_amax")
    nc.vector.tensor_reduce(
        out=amax[:], in_=src3, axis=AX.X, op=ALU.max, apply_absolute_value=True
    )
    t1 = work.tile([128, nblk], F32, tag="q_t1")
    nc.vector.tensor_scalar(
        out=t1[:], in0=amax[:], scalar1=float(np.float32(1.0 / 6.0)), scalar2=gs,
        op0=ALU.mult, op1=ALU.mult,
    )
    sc8 = work.tile([128, nblk], F32, tag="q_sc8")
    nc.vector._custom_dve(
        OP_E4M3, out=sc8[:], in0=t1[:], in1=c448_col,
        s0=VELT_E4M3, s1=MAGIC_E4M3, imm2=TH_E4M3,
    )
    r = work.tile([128, nblk], F32, tag="q_r")
    nc.vector.reciprocal(r[:], sc8[:])
    r2 = work.tile([128, nblk], F32, tag="q_r2")
    nc.vector.tensor_scalar(
        out=r2[:], in0=r[:], scalar1=gs, scalar2=1e38,
        op0=ALU.mult, op1=ALU.min,
    )
    pp = work.tile([128, n], F32, tag="q_pp")
    pp3 = pp[:].rearrange("p (b s) -> p b s", s=16)
    r2b = r2[:].unsqueeze(-1).broadcast_to([128, nblk, 16])
    nc.vector._custom_dve(
        OP_CLIP_SUBNORM, out=pp3, in0=src3, in1=r2b, s0=6.0, s1=MAGIC_E2M1,
    )
    sc8b = sc8[:].unsqueeze(-1).broadcast_to([128, nblk, 16])
    out3 = out_bf16.rearrange("p (b s) -> p b s", s=16)
    nc.vector._custom_dve(OP_VELT_SCALE, out=out3, in0=pp3, in1=sc8b, s0=VELT_E2M1)


# ---------------------------------------------------------------- program


def build_program(gs_x, gs_gw, gs_uw, gs_dw, gs_h):
    gs_x, gs_gw, gs_uw, gs_dw, gs_h = (
        np.float32(gs_x), np.float32(gs_gw), np.float32(gs_uw),
        np.float32(gs_dw), np.float32(gs_h),
    )
    s_gate = float(np.float32(1.0) / np.float32(gs_x * gs_gw))
    s_up = float(np.float32(1.0) / np.float32(gs_x * gs_uw))
    s_down = float(np.float32(1.0) / np.float32(gs_h * gs_dw))

    nc = bacc.Bacc("TRN2", num_devices=NCORES, debug=False)
    x_in = nc.dram_tensor("x_slice", [T_LOC, H], F32, kind="ExternalInput")
    gw_in = nc.dram_tensor("gw_slice", [I_SH, H], F32, kind="ExternalInput")
    uw_in = nc.dram_tensor("uw_slice", [I_SH, H], F32, kind="ExternalInput")
    dw_in = nc.dram_tensor("dw_slice", [HO_SH, I], F32, kind="ExternalInput")
    out_d = nc.dram_tensor("out_slice", [T_LOC, H], F32, kind="ExternalOutput")

    RG = [list(range(NCORES))]
    NT = T_LOC // 128            # 8 token tiles
    NHT = H // 128               # 8 contraction tiles for gate/up
    NCH = I // 512               # 8 I-chunks of 512
    NB = I // 128                # 32 hidden column blocks

    with tile.TileContext(nc) as tc:
        with (
            tc.tile_pool(name="dram", bufs=1, space="DRAM") as dpool,
            tc.tile_pool(name="const", bufs=1) as cpool,
            tc.tile_pool(name="xt", bufs=1) as xtpool,
        ):
            # DRAM staging: local quant shards + gathered weights.
            # guw_loc rows 0-511 = gate shard, 512-1023 = up shard, so ONE
            # AllGather delivers both with each rank's contribution
            # contiguous: chunk k of phase C = core k's I-shard, natural
            # I order.
            guw_loc = dpool.tile([2 * I_SH, H], BF16)
            dw_loc = dpool.tile([HO_SH, I], BF16)
            guw_g = dpool.tile([2 * I_SH * NCORES, H], BF16, addr_space="Shared")
            dw_g = dpool.tile([H, I], BF16, addr_space="Shared")
            xq_d = dpool.tile([T_LOC, H], BF16)
            hq_d = dpool.tile([T_LOC, I], BF16)

            c448 = cpool.tile([128, 1], F32)
            nc.vector.memset(c448[:], 448.0)

            # xqT[128h, ht, tok]  (resident through phase C)
            xqT = xtpool.tile([128, NHT, T_LOC], BF16)

            # ---- Phase A/B: quantize weight stripes + x, kick striped AGs
            with (
                tc.tile_pool(name="raw", bufs=4) as raw,
                tc.tile_pool(name="qout", bufs=4) as qout,
                tc.tile_pool(name="workAB", bufs=3) as workAB,
            ):
                def quant_stripe(src, dst_loc, row0, cols, gsw):
                    wt = raw.tile([128, cols], F32, tag="raw")
                    nc.scalar.dma_start(wt[:], src[row0:row0 + 128, :])
                    wq = qout.tile([128, cols], BF16, tag="qout")
                    quantize_tile(nc, workAB, wt[:], wq[:], cols, gsw, c448[:])
                    nc.scalar.dma_start(dst_loc, wq[:])

                # gate+up shards into the combined tile, then one AllGather
                for j in range(NSTRIPE):
                    quant_stripe(gw_in, guw_loc[j * 128:(j + 1) * 128, :],
                                 j * 128, H, gs_gw)
                for j in range(NSTRIPE):
                    quant_stripe(uw_in,
                                 guw_loc[I_SH + j * 128:I_SH + (j + 1) * 128, :],
                                 j * 128, H, gs_uw)
                nc.gpsimd.collective_compute(
                    "AllGather", ALU.bypass, replica_groups=RG,
                    ins=[guw_loc[:]], outs=[guw_g[:]],
                )

                # x: quantize per 128-token tile, stage via DRAM
                for t in range(NT):
                    xt = raw.tile([128, H], F32, tag="raw")
                    nc.scalar.dma_start(xt[:], x_in[t * 128:(t + 1) * 128, :])
                    xq = qout.tile([128, H], BF16, tag="qout")
                    quantize_tile(nc, workAB, xt[:], xq[:], H, gs_x, c448[:])
                    nc.scalar.dma_start(xq_d[t * 128:(t + 1) * 128, :], xq[:])
                # one big single-writer transpose per ht (simple dep graph:
                # each matmul lhsT read falls inside exactly one writer)
                for ht in range(NHT):
                    nc.sync.dma_start_transpose(
                        xqT[:, ht, :], xq_d[:, ht * 128:(ht + 1) * 128]
                    )

                # down weight shard: quantize in H-sized column pieces
                for p in range(I // H):
                    dwt = raw.tile([128, H], F32, tag="raw")
                    nc.scalar.dma_start(dwt[:], dw_in[:, p * H:(p + 1) * H])
                    dwq = qout.tile([128, H], BF16, tag="qout")
                    quantize_tile(nc, workAB, dwt[:], dwq[:], H, gs_dw, c448[:])
                    nc.scalar.dma_start(dw_loc[:, p * H:(p + 1) * H], dwq[:])
                nc.gpsimd.collective_compute(
                    "AllGather", ALU.bypass, replica_groups=RG,
                    ins=[dw_loc[:]], outs=[dw_g[:]],
                )

            # resident transposed hidden + down weights (filled later)
            with (
                tc.tile_pool(name="hqt", bufs=1) as hqtpool,
                tc.tile_pool(name="dwt", bufs=1) as dwtpool,
                tc.tile_pool(name="wstr", bufs=2) as wstr,
                tc.tile_pool(name="psgu", bufs=2, space="PSUM") as psgu,
                tc.tile_pool(name="hwork", bufs=2) as hwork,
                tc.tile_pool(name="workC", bufs=2) as workC,
                tc.tile_pool(name="hqout", bufs=2) as hqout,
                tc.tile_pool(name="pso", bufs=2, space="PSUM") as pso,
                tc.tile_pool(name="obuf", bufs=2) as obuf,
            ):
                hqT = hqtpool.tile([128, NB, T_LOC], BF16)
                dwT = dwtpool.tile([128, NB, H], BF16)

                # ---- Phase C: gate/up matmuls + SwiGLU + hidden quant
                # weight-chunk transposes are emitted one chunk ahead so
                # they overlap the previous chunk's matmuls on the Sync
                # queue instead of queueing behind its hq stores
                wtiles = []

                def emit_chunk_transposes(k):
                    gwT = wstr.tile([128, NHT, 512], BF16, tag="gwT")
                    uwT = wstr.tile([128, NHT, 512], BF16, tag="uwT")
                    g0 = k * 2 * I_SH
                    for ht in range(NHT):
                        nc.sync.dma_start_transpose(
                            gwT[:, ht, :],
                            guw_g[g0:g0 + 512, ht * 128:(ht + 1) * 128],
                        )
                        nc.sync.dma_start_transpose(
                            uwT[:, ht, :],
                            guw_g[g0 + I_SH:g0 + I_SH + 512,
                                  ht * 128:(ht + 1) * 128],
                        )
                    wtiles.append((gwT, uwT))

                emit_chunk_transposes(0)
                for k in range(NCH):
                    if k + 1 < NCH:
                        emit_chunk_transposes(k + 1)
                    gwT, uwT = wtiles[k]
                    if k == 5:
                        # down-weight transposes (natural I order); AG_dw
                        # has landed by now, phase D needs them much later
                        for b in range(NB):
                            nc.sync.dma_start_transpose(
                                dwT[:, b, :], dw_g[:, b * 128:(b + 1) * 128]
                            )
                    for tch in range(NT):
                        pg = psgu.tile([128, 512], F32, tag="pg")
                        pu = psgu.tile([128, 512], F32, tag="pu")
                        for ht in range(NHT):
                            lhsT = xqT[:, ht, tch * 128:(tch + 1) * 128]
                            nc.tensor.matmul(
                                pg[:], lhsT, gwT[:, ht, :],
                                start=(ht == 0), stop=(ht == NHT - 1),
                            )
                            nc.tensor.matmul(
                                pu[:], lhsT, uwT[:, ht, :],
                                start=(ht == 0), stop=(ht == NHT - 1),
                            )
                        sil = hwork.tile([128, 512], F32, tag="sil")
                        nc.scalar.activation(sil[:], pg[:], AF.Silu, scale=s_gate)
                        htr = hwork.tile([128, 512], F32, tag="htr")
                        nc.vector.scalar_tensor_tensor(
                            out=htr[:], in0=sil[:], scalar=s_up, in1=pu[:],
                            op0=ALU.mult, op1=ALU.mult,
                        )
                        hq = hqout.tile([128, 512], BF16, tag="hq")
                        quantize_tile(nc, workC, htr[:], hq[:], 512, gs_h, c448[:])
                        nc.sync.dma_start(
                            hq_d[tch * 128:(tch + 1) * 128,
                                 k * 512:(k + 1) * 512],
                            hq[:],
                        )
                    # this chunk's hidden columns transpose into resident
                    # hqT as soon as its stores land
                    for b in range(k * 4, k * 4 + 4):
                        nc.sync.dma_start_transpose(
                            hqT[:, b, :], hq_d[:, b * 128:(b + 1) * 128]
                        )

                # ---- Phase D: down matmul + output scale
                for tch in range(NT):
                    po = pso.tile([128, H], F32, tag="po")
                    for b in range(NB):
                        lhsT = hqT[:, b, tch * 128:(tch + 1) * 128]
                        nc.tensor.matmul(
                            po[:, 0:512], lhsT, dwT[:, b, 0:512],
                            start=(b == 0), stop=(b == NB - 1),
                        )
                        nc.tensor.matmul(
                            po[:, 512:1024], lhsT, dwT[:, b, 512:1024],
                            start=(b == 0), stop=(b == NB - 1),
                        )
                    ob = obuf.tile([128, H], F32, tag="ob")
                    nc.scalar.activation(ob[:], po[:], AF.Copy, scale=s_down)
                    nc.scalar.dma_start(
                        out_d[tch * 128:(tch + 1) * 128, :], ob[:]
                    )

    nc.finalize()
    return nc


_PROG_CACHE = {}
TRACE = False          # set by test.py to capture an NTFF profile
LAST_EXEC_NS = None
LAST_RESULTS = None


def kernel(x, gate_w, up_w, down_w, s_in, s_in_down):
    x = np.ascontiguousarray(x, dtype=np.float32)
    gate_w = np.ascontiguousarray(gate_w, dtype=np.float32)
    up_w = np.ascontiguousarray(up_w, dtype=np.float32)
    down_w = np.ascontiguousarray(down_w, dtype=np.float32)
    gs_x = np.float32(np.asarray(s_in).reshape(-1)[0])
    gs_h = np.float32(np.asarray(s_in_down).reshape(-1)[0])
    FM = np.float32(448.0 * 6.0)
    gs_gw = np.float32(FM / np.abs(gate_w).max())
    gs_uw = np.float32(FM / np.abs(up_w).max())
    gs_dw = np.float32(FM / np.abs(down_w).max())

    key = tuple(float(v) for v in (gs_x, gs_gw, gs_uw, gs_dw, gs_h))
    if key not in _PROG_CACHE:
        _PROG_CACHE.clear()
        _PROG_CACHE[key] = build_program(*key)
    nc = _PROG_CACHE[key]

    xf = x.reshape(T, H)
    in_maps = []
    for c in range(NCORES):
        in_maps.append({
            "x_slice": np.ascontiguousarray(xf[c * T_LOC:(c + 1) * T_LOC]),
            "gw_slice": np.ascontiguousarray(gate_w[c * I_SH:(c + 1) * I_SH]),
            "uw_slice": np.ascontiguousarray(up_w[c * I_SH:(c + 1) * I_SH]),
            "dw_slice": np.ascontiguousarray(down_w[c * HO_SH:(c + 1) * HO_SH]),
        })
    global LAST_EXEC_NS, LAST_RESULTS
    res = run_bass_kernel_spmd(
        nc, in_maps, core_ids=list(range(NCORES)), trace=TRACE
    )
    LAST_EXEC_NS = res.exec_time_ns
    LAST_RESULTS = res
    out = np.concatenate([r["out_slice"] for r in res.results], axis=0)
    return out.reshape(B, S, H).astype(np.float32)


if __name__ == "__main__":
    rng = np.random.default_rng(0)
    inputs = dict(
        x=rng.standard_normal((B, S, H), dtype=np.float32),
        gate_w=0.05 * rng.standard_normal((I, H), dtype=np.float32),
        up_w=0.05 * rng.standard_normal((I, H), dtype=np.float32),
        down_w=0.05 * rng.standard_normal((H, I), dtype=np.float32),
        s_in=np.array([700.0], dtype=np.float32),
        s_in_down=np.array([800.0], dtype=np.float32),
    )
    out = kernel(**inputs)
    print("kernel output", out.shape, out.dtype, np.abs(out).max())
